# revision 1
# baseline (speedup 1.0000x reference)
"""Bilateral filter 3x3 (sigma_space = sigma_color = 0.8) on 8 TRN2 NeuronCores.

Strategy (per core = one batch image [3, 512, 512] fp32):
  out = c + A/den with the color-normalization cancelled:
    den(x) = ws0 + sum_{k in HP} [G_k(x) + G_k(x-k)]
    A(x)   =       sum_{k in HP} [H_k(x) - H_k(x-k)]
  where HP = {E=(0,1), S=(1,0), SE=(1,1), SW=(1,-1)},
    D_k = p~(x+k) - p(x),  G_k = ws_k * exp(-D_k^2 / (2 s^2)),  H_k = D_k * G_k.
  Shifted terms G_k(x-k)/H_k(x-k) are accumulated on the TensorEngine with
  shift-band matmuls (fp16, 1 cyc/row) into PSUM; row seams across 128-row
  tiles use a selector band against the previous tile's G/H; image boundaries
  use reflect-mirror identities applied in the D domain (D odd, G even).
  Emission is software-pipelined (evac of tile t-1 between tile t's subs and
  muls) so each engine's FIFO stays fed; two subs ride GPSIMD; loads/stores/
  consts use separate DMA queues (sync/scalar/gpsimd) to avoid head-of-line
  blocking.

Layout: partition = image rows (4 tiles x 128 rows), free = (channel, width)
with 1-col halo pads (width 514).
"""
import math
import numpy as np
from contextlib import ExitStack

import concourse.bacc as bacc
import concourse.tile as tile
from concourse import mybir
from concourse.bass_utils import run_bass_kernel_spmd

F32 = mybir.dt.float32
F32R = mybir.dt.float32r
F16 = mybir.dt.float16
MM_DT = F16                 # accumulation-path dtype: F16 (1cyc/row) or F32R
MM_NP = np.float16 if MM_DT == F16 else np.float32
AF = mybir.ActivationFunctionType

C, H, W = 3, 512, 512
P = 128                      # partitions per row-tile
NT = H // P                  # 4 row-tiles
WP = W + 2                   # col-padded width

SIG = 0.8
TWO_SIG2 = 2.0 * SIG * SIG   # 1.28
SCALE_SQ = 1.0 / math.sqrt(TWO_SIG2)
_w1 = math.exp(-1.0 / TWO_SIG2)
_norm = (1.0 + 2.0 * _w1) ** 2
WS0 = 1.0 / _norm            # center weight
WS_E = _w1 / _norm           # edge
WS_K = _w1 * _w1 / _norm     # corner
LNW_E = math.log(WS_E)
LNW_K = math.log(WS_K)

# band order in the packed const tensor
BAND_NAMES = ["b_ws0", "b_i", "b_is", "b_s", "b_ni", "b_ns", "b_ins",
              "b_sel", "b_nsel", "b_sel0"]


def _bands_np():
    I = np.eye(P, dtype=np.float32)
    S = np.zeros((P, P), np.float32)
    for m in range(1, P):
        S[m - 1, m] = 1.0          # lhsT[p, m]: out row m <- in row m-1
    sel = np.zeros((P, P), np.float32)
    sel[P - 1, 0] = 1.0            # out row 0 <- in row 127 (prev tile)
    sel0 = np.zeros((P, P), np.float32)
    sel0[0, 0] = 1.0               # out row 0 <- in row 0 (top mirror)
    d = {"b_ws0": WS0 * I, "b_i": I, "b_is": I + S, "b_s": S, "b_ni": -I,
         "b_ns": -S, "b_ins": I - S, "b_sel": sel, "b_nsel": -sel,
         "b_sel0": sel0}
    return np.stack([d[k] for k in BAND_NAMES], axis=1)  # [P, 10, P]


def build():
    nc = bacc.Bacc("TRN2", target_bir_lowering=False, debug=False)
    x_d = nc.dram_tensor("x", [C, H, W], F32, kind="ExternalInput")
    y_d = nc.dram_tensor("y", [C, H, W], F32, kind="ExternalOutput")

    bands_d = nc.inline_tensor(_bands_np().astype(MM_NP), "bands")
    # per-partition exp biases: col 0 = ln(ws_edge), col 1 = ln(ws_corner)
    bias_np = np.stack([np.full(P, LNW_E, np.float32),
                        np.full(P, LNW_K, np.float32)], axis=1)  # [P, 2]
    bias_d = nc.inline_tensor(bias_np, "lnw")
    ones_d = nc.inline_tensor(np.ones((P, W), MM_NP), "ones_c")

    xh = x_d.ap().rearrange("c h w -> h c w")   # partition = image row
    yh = y_d.ap().rearrange("c h w -> h c w")

    with tile.TileContext(nc) as tc, ExitStack() as ctx:
        const = ctx.enter_context(tc.tile_pool(name="const", bufs=1))
        pp = ctx.enter_context(tc.tile_pool(name="pp", bufs=3))
        dp = ctx.enter_context(tc.tile_pool(name="dp", bufs=2))
        gp = ctx.enter_context(tc.tile_pool(name="gp", bufs=2))
        hp = ctx.enter_context(tc.tile_pool(name="hp", bufs=2))
        fin = ctx.enter_context(tc.tile_pool(name="fin", bufs=2))
        sqp = ctx.enter_context(tc.tile_pool(name="sqp", bufs=2))
        psp = ctx.enter_context(tc.tile_pool(name="psp", bufs=1, space="PSUM"))

        # --- constants ---
        bands_t = const.tile([P, len(BAND_NAMES), P], MM_DT, tag="bands")
        nc.gpsimd.dma_start(out=bands_t, in_=bands_d.ap())
        B = {k: bands_t[:, i, :] for i, k in enumerate(BAND_NAMES)}
        ones = const.tile([P, W], MM_DT, tag="ones")
        nc.gpsimd.dma_start(out=ones, in_=ones_d.ap())
        bias_t = const.tile([P, 2], F32, tag="bias")
        nc.gpsimd.dma_start(out=bias_t, in_=bias_d.ap())
        lnw_e, lnw_k = bias_t[:, 0:1], bias_t[:, 1:2]
        # absorb the bias DMA wait on ACT once (activation has 1 wait slot)
        scratch = const.tile([P, 2], F32, tag="scratch")
        nc.scalar.copy(scratch, bias_t)
        # absorb the bands DMA wait on PE once
        ps_scr = psp.tile([P, W], F32, tag="den0", bufs=2, name="ps_scr")
        nc.tensor.matmul(ps_scr[:, :P], B["b_i"], B["b_i"], start=True, stop=True)

        prev_g = None
        prev_h = None
        prev_evac = None   # (den_ps, a_ps, pmid, r0) of previous tile
        for t in range(NT + 1):
            if t < NT:
                r0 = t * P
                # --- load P_mid (rows r0..r0+127), P_dn (rows r0+1..r0+128) ---
                pmid = pp.tile([P, C, WP], F32, tag="pmid", name=f"pmid_{t}")
                nc.sync.dma_start(out=pmid[:, :, 1 : W + 1], in_=xh[r0 : r0 + P])
                pdn = pp.tile([P, C, WP], F32, tag="pdn", name=f"pdn_{t}")
                if t < NT - 1:
                    nc.sync.dma_start(out=pdn[:, :, 1 : W + 1], in_=xh[r0 + 1 : r0 + P + 1])
                else:
                    nc.sync.dma_start(out=pdn[: P - 1, :, 1 : W + 1], in_=xh[r0 + 1 : H])
                    # reflect: image row 512 -> row 510 (SWDGE; off the Sync queue)
                    nc.gpsimd.dma_start(out=pdn[P - 1 : P, :, 1 : W + 1], in_=xh[H - 2 : H - 1])
                # col halos (reflect): buf col0 <- image col1 (=buf col2);
                # buf col513 <- image col510 (=buf col511)   (GpSimd: DVE is hot)
                for pt in (pmid, pdn):
                    nc.vector.tensor_copy(pt[:, :, 0:1], pt[:, :, 2:3])
                    nc.vector.tensor_copy(pt[:, :, WP - 1 : WP], pt[:, :, WP - 3 : WP - 2])

                cen = pmid[:, :, 1 : W + 1]

                # --- D_k = P(x+k) - P(x), col-padded (DVE) ---
                d = {}
                for name in ("e", "s", "se", "sw"):
                    d[name] = dp.tile([P, C, WP], F32, tag=f"d_{name}", name=f"d_{name}_{t}")
                nc.vector.tensor_sub(d["e"][:, :, 1 : W + 1], pmid[:, :, 2 : W + 2], cen)
                se_eng = nc.vector if t == 0 else nc.gpsimd
                se_eng.tensor_sub(d["s"][:, :, 1 : W + 1], pdn[:, :, 1 : W + 1], cen)
                nc.vector.tensor_sub(d["se"][:, :, 1 : W + 1], pdn[:, :, 2 : W + 2], cen)
                se_eng.tensor_sub(d["sw"][:, :, 1 : W + 1], pdn[:, :, 0:W], cen)
                # zero both pad cols of each D buffer once (2 rotating buffers),
                # then overwrite consumed pads with mirrors:
                #   D_E(h,-1) = -D_E(h,0); D_SE(h,-1) = D_SW(h,1);
                #   D_SW(h,W) = D_SE(h,W-2)        (GpSimd)
                if t <= 1:
                    for name in ("e", "s", "se", "sw"):
                        nc.vector.memset(d[name][:, :, 0:1], 0.0)
                        nc.vector.memset(d[name][:, :, WP - 1 : WP], 0.0)
                nc.scalar.mul(d["e"][:, :, 0:1], d["e"][:, :, 1:2], -1.0)
                nc.scalar.copy(d["se"][:, :, 0:1], d["sw"][:, :, 2:3])
                nc.scalar.copy(d["sw"][:, :, WP - 1 : WP], d["se"][:, :, WP - 3 : WP - 2])

            if t >= 1:
                # --- evac of previous tile: y = c + A * (1/den)  (DVE) ---
                pden, pa, ppm, pr0 = prev_evac
                yt = fin.tile([P, C, W], F32, tag="yt", name=f"yt_{t-1}")
                for c in range(C):
                    rec = fin.tile([P, W], F32, tag="rec", name=f"rec{c}_{t-1}")
                    nc.vector.reciprocal_approx_fast(out=rec, in_=pden[c])
                    t1 = fin.tile([P, W], F32, tag="t1", name=f"t1{c}_{t-1}")
                    nc.vector.tensor_mul(t1, pa[c], rec)
                    nc.vector.tensor_add(yt[:, c, :], t1, ppm[:, c, 1 : W + 1])
                nc.scalar.dma_start(out=yh[pr0 : pr0 + P], in_=yt)

            if t < NT:
                # --- G_k = ws_k * exp(-D^2/(2s^2)), full width (ACT) ---
                g, h = {}, {}
                for name, lnw in (("e", lnw_e), ("s", lnw_e), ("se", lnw_k), ("sw", lnw_k)):
                    gk = gp.tile([P, C, WP], MM_DT, tag=f"g_{name}", name=f"g_{name}_{t}")
                    sq = sqp.tile([P, C, WP], F32, tag="sq", name=f"sq_{name}_{t}")
                    nc.scalar.activation(sq, d[name], AF.Square,
                                         bias=0.0, scale=SCALE_SQ)
                    nc.scalar.activation(gk, sq, AF.Exp, bias=lnw,
                                         scale=-1.0)
                    g[name] = gk
                # --- H_k = D_k * G_k, full width (DVE, fp16 out) ---
                for name in ("e", "s", "se", "sw"):
                    hk = hp.tile([P, C, WP], MM_DT, tag=f"h_{name}", name=f"h_{name}_{t}")
                    nc.vector.tensor_mul(hk, d[name], g[name])
                    h[name] = hk

                # --- PSUM accumulation chains (PE, fp16) ---
                den_ps = [psp.tile([P, W], F32, tag=f"den{c}", name=f"den{c}_{t}",
                                    bufs=2 if c <= 1 else 1) for c in range(C)]
                a_ps = [psp.tile([P, W], F32, tag=f"a{c}", name=f"a{c}_{t}")
                        for c in range(C)]
                for c in range(C):
                    dn = den_ps[c]
                    gE, gS, gSE, gSW = (g[n][:, c, :] for n in ("e", "s", "se", "sw"))
                    hE, hS, hSE, hSW = (h[n][:, c, :] for n in ("e", "s", "se", "sw"))
                    J0, J1, J2 = slice(0, W), slice(1, W + 1), slice(2, W + 2)
                    # den chain
                    nc.tensor.matmul(dn, B["b_ws0"], ones, start=True, stop=False)
                    nc.tensor.matmul(dn, B["b_i"], gE[:, J1], start=False, stop=False)
                    nc.tensor.matmul(dn, B["b_i"], gE[:, J0], start=False, stop=False)
                    nc.tensor.matmul(dn, B["b_is"], gS[:, J1], start=False, stop=False)
                    nc.tensor.matmul(dn, B["b_i"], gSE[:, J1], start=False, stop=False)
                    nc.tensor.matmul(dn, B["b_s"], gSE[:, J0], start=False, stop=False)
                    nc.tensor.matmul(dn, B["b_i"], gSW[:, J1], start=False, stop=False)
                    nc.tensor.matmul(dn, B["b_s"], gSW[:, J2], start=False, stop=False)
                    if t == 0:
                        nc.tensor.matmul(dn, B["b_sel0"], gS[:, J1], start=False, stop=False)
                        nc.tensor.matmul(dn, B["b_sel0"], gSE[:, J1], start=False, stop=False)
                        nc.tensor.matmul(dn, B["b_sel0"], gSW[:, J1], start=False, stop=True)
                    else:
                        pgS, pgSE, pgSW = (prev_g[n][:, c, :] for n in ("s", "se", "sw"))
                        nc.tensor.matmul(dn, B["b_sel"], pgS[:, J1], start=False, stop=False)
                        nc.tensor.matmul(dn, B["b_sel"], pgSE[:, J0], start=False, stop=False)
                        nc.tensor.matmul(dn, B["b_sel"], pgSW[:, J2], start=False, stop=True)
                    # A chain
                    an = a_ps[c]
                    nc.tensor.matmul(an, B["b_i"], hE[:, J1], start=True, stop=False)
                    nc.tensor.matmul(an, B["b_ni"], hE[:, J0], start=False, stop=False)
                    nc.tensor.matmul(an, B["b_ins"], hS[:, J1], start=False, stop=False)
                    nc.tensor.matmul(an, B["b_i"], hSE[:, J1], start=False, stop=False)
                    nc.tensor.matmul(an, B["b_ns"], hSE[:, J0], start=False, stop=False)
                    nc.tensor.matmul(an, B["b_i"], hSW[:, J1], start=False, stop=False)
                    nc.tensor.matmul(an, B["b_ns"], hSW[:, J2], start=False, stop=False)
                    if t == 0:
                        nc.tensor.matmul(an, B["b_sel0"], hS[:, J1], start=False, stop=False)
                        nc.tensor.matmul(an, B["b_sel0"], hSE[:, J1], start=False, stop=False)
                        nc.tensor.matmul(an, B["b_sel0"], hSW[:, J1], start=False, stop=True)
                    else:
                        phS, phSE, phSW = (prev_h[n][:, c, :] for n in ("s", "se", "sw"))
                        nc.tensor.matmul(an, B["b_nsel"], phS[:, J1], start=False, stop=False)
                        nc.tensor.matmul(an, B["b_nsel"], phSE[:, J0], start=False, stop=False)
                        nc.tensor.matmul(an, B["b_nsel"], phSW[:, J2], start=False, stop=True)

                prev_g, prev_h = g, h
                prev_evac = (den_ps, a_ps, pmid, r0)

    nc.compile()
    return nc


_NC_CACHE = None


def _get_nc():
    global _NC_CACHE
    if _NC_CACHE is None:
        _NC_CACHE = build()
    return _NC_CACHE


def kernel(batch_img: np.ndarray) -> np.ndarray:
    assert batch_img.shape == (8, C, H, W), batch_img.shape
    x = np.ascontiguousarray(np.asarray(batch_img, dtype=np.float32))
    nc = _get_nc()
    in_maps = [{"x": x[b]} for b in range(8)]
    r = run_bass_kernel_spmd(nc, in_maps, core_ids=list(range(8)))
    out = np.stack([r.results[b]["y"] for b in range(8)], axis=0)
    return out.astype(np.float32)


if __name__ == "__main__":
    rng = np.random.default_rng(0)
    img = rng.random((8, C, H, W), np.float32)
    y = kernel(img)
    print("ran ok", y.shape, y.dtype)



# revision 3
# speedup vs baseline: 1.2171x; 1.2171x over previous
"""Bilateral filter 3x3 (sigma_space = sigma_color = 0.8) on 8 TRN2 NeuronCores.

Strategy (per core = one batch image [3, 512, 512]):
  out = c + A/den with the color-normalization cancelled:
    den(x) = ws0 + sum_{k in HP} [G_k(x) + G_k(x-k)]
    A(x)   =       sum_{k in HP} [H_k(x) - H_k(x-k)]
  where HP = {E=(0,1), S=(1,0), SE=(1,1), SW=(1,-1)},
    D_k = p~(x+k) - p(x),  G_k = ws_k * exp(-D_k^2 / (2 s^2)),  H_k = D_k * G_k.

v2 perf rework vs the f32 baseline:
  - fp16 everywhere on-chip (input converted + transposed to [H,C,W] fp16 on
    the host; output fp16, upcast on host).  DVE tensor_tensor runs in 2x_1P
    packed mode; HBM traffic halves; PE streams fp16 at 1 cyc/row.
  - width-522 col-padded layout so every bulk DVE op is 4B-aligned (2x mode).
  - subs with odd-aligned operands (E/SE/SW) ride GPSIMD; the aligned one (S)
    plus all H muls ride DVE at 2x.
  - evacuation: A copied PSUM->SBUF (DVE 2x), then one fused custom DVE op
    t8 = reciprocal1(den + ws0) * A  (bit-trick seed + 1 Newton step, 7 ALU
    stages), then one packed fp16 add  y = t8 + center.  This kills the
    separate reciprocal pass, the separate ws0*ones PE pass, and runs the
    final add at 2x.
  - Shifted terms G_k(x-k)/H_k(x-k) still accumulate on the TensorEngine with
    shift-band matmuls into PSUM; row seams across 128-row tiles use selector
    bands against the previous tile's G/H; image boundaries use
    reflect-mirror identities applied in the D domain (D odd, G even).

Layout: partition = image rows (4 tiles x 128 rows), free = (channel, width)
with pads: WP=522 cols; col 1 = left halo, cols 2..513 = image, col 514 =
right halo; cols 0 and >=515 are zeroed/unused (522 keeps channel starts
8-byte aligned).  J0 = cols 1..512, J1 = 2..513, J2 = 3..514.
"""
import math
import numpy as np
from contextlib import ExitStack

import concourse.bacc as bacc
import concourse.tile as tile
from concourse import mybir
from concourse.bass_utils import run_bass_kernel_spmd

F32 = mybir.dt.float32
F16 = mybir.dt.float16
MM_DT = F16
MM_NP = np.float16
AF = mybir.ActivationFunctionType

C, H, W = 3, 512, 512
P = 128                      # partitions per row-tile
NT = H // P                  # 4 row-tiles
WP = 522                     # col-padded width (really use 516; 522 = mult of 6 hmm keep simple)
IM0 = 2                      # first image column inside a padded row
J0 = IM0 - 1                 # slice starts
J1 = IM0
J2 = IM0 + 1

SIG = 0.8
TWO_SIG2 = 2.0 * SIG * SIG   # 1.28
SCALE_SQ = 1.0 / math.sqrt(TWO_SIG2)
_w1 = math.exp(-1.0 / TWO_SIG2)
_norm = (1.0 + 2.0 * _w1) ** 2
WS0 = 1.0 / _norm            # center weight
WS_E = _w1 / _norm           # edge
WS_K = _w1 * _w1 / _norm     # corner
LNW_E = math.log(WS_E)
LNW_K = math.log(WS_K)

# band order in the packed const tensor
BAND_NAMES = ["b_i", "b_is", "b_s", "b_ni", "b_ns", "b_ins",
              "b_sel", "b_nsel", "b_sel0"]


def _bands_np():
    I = np.eye(P, dtype=np.float32)
    S = np.zeros((P, P), np.float32)
    for m in range(1, P):
        S[m - 1, m] = 1.0          # lhsT[p, m]: out row m <- in row m-1
    sel = np.zeros((P, P), np.float32)
    sel[P - 1, 0] = 1.0            # out row 0 <- in row 127 (prev tile)
    sel0 = np.zeros((P, P), np.float32)
    sel0[0, 0] = 1.0               # out row 0 <- in row 0 (top mirror)
    d = {"b_i": I, "b_is": I + S, "b_s": S, "b_ni": -I,
         "b_ns": -S, "b_ins": I - S, "b_sel": sel, "b_nsel": -sel,
         "b_sel0": sel0}
    return np.stack([d[k] for k in BAND_NAMES], axis=1)  # [P, 9, P]


# ------------- custom DVE op: t8 = recip1(den + ws0) * A -------------------
_RECIP_OP = None


def _get_recip_op():
    """out = y1 * in1 with y1 = one-Newton-step reciprocal of (in0 + s0).

    Seed: bitcast(~x) * c_cheb0 (exponent-flip trick, ~6% rel err), one NR
    pass y0*(c_cheb1 - x*y0) -> ~0.4% max rel err, plenty for the 2e-2
    tolerance (|A/den| <= ~0.3).
    """
    global _RECIP_OP
    if _RECIP_OP is not None:
        return _RECIP_OP
    from concourse import dve_ops as dvo
    from concourse.dve_spec import Spec, Src0, Src1, C0, C1, C2, AluOp, Bin, lower
    from concourse.dve_uop import DveOpSpec

    name = "RECIP1MUL_WS0_ANT"
    xs = Src0 + C0
    nx = Bin(AluOp.BITWISE_NOT, xs, xs)
    y0 = nx * C1
    y1 = y0 * (C2 - xs * y0)
    body = y1 * Src1

    def _ref(in0, in1, c0, c1, c2):
        xs = np.ascontiguousarray(in0.astype(np.float32) + np.float32(c0))
        nx = (~xs.view(np.int32)).view(np.float32)
        y0 = nx * np.float32(c1)
        y1 = y0 * (np.float32(c2) - xs * y0)
        return (y1 * in1.astype(np.float32)).astype(np.float32)

    spec = Spec(body=body, reference=_ref)
    shas = {}
    for ver in ("v3", "v4"):
        try:
            s = DveOpSpec(name=name, opcode=None, uops=lower(spec, ver=ver),
                          rd1_en=True)
            shas[ver] = s.sha(ver)
        except Exception:
            pass
    op = dvo.DveOp(name, spec, subdim=False, uops_sha=shas)
    if name not in dvo._SUB_OPCODE_FOR_NAME:
        dvo.OPS.append(op)
        dvo._SUB_OPCODE_FOR_NAME[name] = dvo._CUSTOM_DVE_ROW_BASE + len(dvo.OPS) - 1
        dvo.CUSTOM_DVE_SPECS[name] = spec
        assert dvo._SUB_OPCODE_FOR_NAME[name] < 0x20
    _RECIP_OP = op
    return op


# Chebyshev-minimax seed constants (same as RECIPROCAL_APPROX_FAST)
_RC0 = -0.23549792
_RC1 = 2.0017324


def build():
    recip_op = _get_recip_op()
    nc = bacc.Bacc("TRN2", target_bir_lowering=False, debug=False)
    x_d = nc.dram_tensor("x", [H, C, W], F16, kind="ExternalInput")
    y_d = nc.dram_tensor("y", [H, C, W], F16, kind="ExternalOutput")

    bands_d = nc.inline_tensor(_bands_np().astype(MM_NP), "bands")
    # per-partition exp biases: col 0 = ln(ws_edge), col 1 = ln(ws_corner)
    bias_np = np.stack([np.full(P, LNW_E, np.float32),
                        np.full(P, LNW_K, np.float32)], axis=1)  # [P, 2]
    bias_d = nc.inline_tensor(bias_np, "lnw")

    xh = x_d.ap()   # [H, C, W], partition = image row
    yh = y_d.ap()

    with tile.TileContext(nc) as tc, ExitStack() as ctx:
        const = ctx.enter_context(tc.tile_pool(name="const", bufs=1))
        pp = ctx.enter_context(tc.tile_pool(name="pp", bufs=3))
        dp = ctx.enter_context(tc.tile_pool(name="dp", bufs=2))
        gp = ctx.enter_context(tc.tile_pool(name="gp", bufs=2))
        hp = ctx.enter_context(tc.tile_pool(name="hp", bufs=2))
        fin = ctx.enter_context(tc.tile_pool(name="fin", bufs=2))
        sqp = ctx.enter_context(tc.tile_pool(name="sqp", bufs=2))
        psp = ctx.enter_context(tc.tile_pool(name="psp", bufs=1, space="PSUM"))

        # --- constants ---
        bands_t = const.tile([P, len(BAND_NAMES), P], MM_DT, tag="bands")
        nc.gpsimd.dma_start(out=bands_t, in_=bands_d.ap())
        B = {k: bands_t[:, i, :] for i, k in enumerate(BAND_NAMES)}
        bias_t = const.tile([P, 2], F32, tag="bias")
        nc.gpsimd.dma_start(out=bias_t, in_=bias_d.ap())
        lnw_e, lnw_k = bias_t[:, 0:1], bias_t[:, 1:2]
        # absorb the bias DMA wait on ACT once (activation has 1 wait slot)
        scratch = const.tile([P, 2], F32, tag="scratch")
        nc.scalar.copy(scratch, bias_t)
        # absorb the bands DMA wait on PE once
        ps_scr = psp.tile([P, W], F32, tag="den0", bufs=2, name="ps_scr")
        nc.tensor.matmul(ps_scr[:, :P], B["b_i"], B["b_i"], start=True, stop=True)

        prev_g = None
        prev_h = None
        prev_evac = None   # (den_ps, a_ps, pmid, r0) of previous tile
        for t in range(NT + 1):
            if t < NT:
                r0 = t * P
                # --- load P_mid (rows r0..r0+127), P_dn (rows r0+1..r0+128) ---
                pmid = pp.tile([P, C, WP], F16, tag="pmid", name=f"pmid_{t}")
                nc.sync.dma_start(out=pmid[:, :, IM0 : IM0 + W], in_=xh[r0 : r0 + P])
                pdn = pp.tile([P, C, WP], F16, tag="pdn", name=f"pdn_{t}")
                if t < NT - 1:
                    nc.sync.dma_start(out=pdn[:, :, IM0 : IM0 + W], in_=xh[r0 + 1 : r0 + P + 1])
                else:
                    nc.sync.dma_start(out=pdn[: P - 1, :, IM0 : IM0 + W], in_=xh[r0 + 1 : H])
                    # reflect: image row 512 -> row 510 (SWDGE; off the Sync queue)
                    nc.gpsimd.dma_start(out=pdn[P - 1 : P, :, IM0 : IM0 + W], in_=xh[H - 2 : H - 1])
                # col halos (reflect): buf col J0 <- image col1 (=buf IM0+1);
                # buf col IM0+W <- image col W-2 (=buf IM0+W-2)
                for pt in (pmid, pdn):
                    nc.vector.tensor_copy(pt[:, :, J0 : J0 + 1], pt[:, :, IM0 + 1 : IM0 + 2])
                    nc.vector.tensor_copy(pt[:, :, IM0 + W : IM0 + W + 1],
                                          pt[:, :, IM0 + W - 2 : IM0 + W - 1])

                cen = pmid[:, :, J1 : J1 + W]

                # --- D_k = P(x+k) - P(x), fp16 (S aligned -> DVE 2x; others GPSIMD) ---
                d = {}
                for name in ("e", "s", "se", "sw"):
                    d[name] = dp.tile([P, C, WP], F16, tag=f"d_{name}", name=f"d_{name}_{t}")
                if t <= 1:
                    # zero every pad col of both rotating buffers once
                    for name in ("e", "s", "se", "sw"):
                        nc.vector.memset(d[name][:, :, 0:J1], 0.0)
                        nc.vector.memset(d[name][:, :, J1 + W : WP], 0.0)
                nc.vector.tensor_sub(d["s"][:, :, J1 : J1 + W], pdn[:, :, J1 : J1 + W], cen)
                nc.gpsimd.tensor_sub(d["e"][:, :, J1 : J1 + W], pmid[:, :, J2 : J2 + W], cen)
                nc.gpsimd.tensor_sub(d["se"][:, :, J1 : J1 + W], pdn[:, :, J2 : J2 + W], cen)
                nc.gpsimd.tensor_sub(d["sw"][:, :, J1 : J1 + W], pdn[:, :, J0 : J0 + W], cen)
                # overwrite consumed pads with mirrors:
                #   D_E(h,-1) = -D_E(h,0); D_SE(h,-1) = D_SW(h,1);
                #   D_SW(h,W) = D_SE(h,W-2)
                nc.scalar.mul(d["e"][:, :, J0 : J0 + 1], d["e"][:, :, J1 : J1 + 1], -1.0)
                nc.scalar.copy(d["se"][:, :, J0 : J0 + 1], d["sw"][:, :, J2 : J2 + 1])
                nc.scalar.copy(d["sw"][:, :, J2 + W - 1 : J2 + W],
                               d["se"][:, :, J2 + W - 3 : J2 + W - 2])

            if t >= 1:
                # --- evac of previous tile: y = c + A * recip1(den + ws0) ---
                pden, pa, ppm, pr0 = prev_evac
                a_sb = fin.tile([P, C, W], F32, tag="a_sb", name=f"a_sb_{t-1}")
                t8 = fin.tile([P, C, W], F16, tag="t8", name=f"t8_{t-1}")
                for c in range(C):
                    nc.vector.tensor_copy(a_sb[:, c, :], pa[c])
                    nc.vector._custom_dve(
                        recip_op, out=t8[:, c, :], in0=pden[c], in1=a_sb[:, c, :],
                        s0=WS0, s1=_RC0, imm2=_RC1)
                yt = fin.tile([P, C, W], F16, tag="yt", name=f"yt_{t-1}")
                nc.vector.tensor_add(yt, t8, ppm[:, :, J1 : J1 + W])
                nc.scalar.dma_start(out=yh[pr0 : pr0 + P], in_=yt)

            if t < NT:
                # --- G_k = ws_k * exp(-D^2/(2s^2)) over the full padded width (ACT) ---
                g, h = {}, {}
                for name, lnw in (("e", lnw_e), ("s", lnw_e), ("se", lnw_k), ("sw", lnw_k)):
                    gk = gp.tile([P, C, WP], MM_DT, tag=f"g_{name}", name=f"g_{name}_{t}")
                    sq = sqp.tile([P, C, WP], F16, tag="sq", name=f"sq_{name}_{t}")
                    nc.scalar.activation(sq, d[name], AF.Square,
                                         bias=0.0, scale=SCALE_SQ)
                    nc.scalar.activation(gk, sq, AF.Exp, bias=lnw,
                                         scale=-1.0)
                    g[name] = gk
                # --- H_k = D_k * G_k (DVE, fp16 2x) ---
                for name in ("e", "s", "se", "sw"):
                    hk = hp.tile([P, C, WP], MM_DT, tag=f"h_{name}", name=f"h_{name}_{t}")
                    nc.vector.tensor_mul(hk, d[name], g[name])
                    h[name] = hk

                # --- PSUM accumulation chains (PE, fp16) ---
                den_ps = [psp.tile([P, W], F32, tag=f"den{c}", name=f"den{c}_{t}",
                                    bufs=2 if c <= 1 else 1) for c in range(C)]
                a_ps = [psp.tile([P, W], F32, tag=f"a{c}", name=f"a{c}_{t}")
                        for c in range(C)]

                def sl(ap, c, j):
                    return ap[:, c, j : j + W]

                for c in range(C):
                    dn = den_ps[c]
                    gE, gS, gSE, gSW = g["e"], g["s"], g["se"], g["sw"]
                    hE, hS, hSE, hSW = h["e"], h["s"], h["se"], h["sw"]
                    # den chain (ws0 is folded into the evac custom op)
                    nc.tensor.matmul(dn, B["b_i"], sl(gE, c, J1), start=True, stop=False)
                    nc.tensor.matmul(dn, B["b_i"], sl(gE, c, J0), start=False, stop=False)
                    nc.tensor.matmul(dn, B["b_is"], sl(gS, c, J1), start=False, stop=False)
                    nc.tensor.matmul(dn, B["b_i"], sl(gSE, c, J1), start=False, stop=False)
                    nc.tensor.matmul(dn, B["b_s"], sl(gSE, c, J0), start=False, stop=False)
                    nc.tensor.matmul(dn, B["b_i"], sl(gSW, c, J1), start=False, stop=False)
                    nc.tensor.matmul(dn, B["b_s"], sl(gSW, c, J2), start=False, stop=False)
                    if t == 0:
                        nc.tensor.matmul(dn, B["b_sel0"], sl(gS, c, J1), start=False, stop=False)
                        nc.tensor.matmul(dn, B["b_sel0"], sl(gSE, c, J1), start=False, stop=False)
                        nc.tensor.matmul(dn, B["b_sel0"], sl(gSW, c, J1), start=False, stop=True)
                    else:
                        pgS, pgSE, pgSW = prev_g["s"], prev_g["se"], prev_g["sw"]
                        nc.tensor.matmul(dn, B["b_sel"], sl(pgS, c, J1), start=False, stop=False)
                        nc.tensor.matmul(dn, B["b_sel"], sl(pgSE, c, J0), start=False, stop=False)
                        nc.tensor.matmul(dn, B["b_sel"], sl(pgSW, c, J2), start=False, stop=True)
                    # A chain
                    an = a_ps[c]
                    nc.tensor.matmul(an, B["b_i"], sl(hE, c, J1), start=True, stop=False)
                    nc.tensor.matmul(an, B["b_ni"], sl(hE, c, J0), start=False, stop=False)
                    nc.tensor.matmul(an, B["b_ins"], sl(hS, c, J1), start=False, stop=False)
                    nc.tensor.matmul(an, B["b_i"], sl(hSE, c, J1), start=False, stop=False)
                    nc.tensor.matmul(an, B["b_ns"], sl(hSE, c, J0), start=False, stop=False)
                    nc.tensor.matmul(an, B["b_i"], sl(hSW, c, J1), start=False, stop=False)
                    nc.tensor.matmul(an, B["b_ns"], sl(hSW, c, J2), start=False, stop=False)
                    if t == 0:
                        nc.tensor.matmul(an, B["b_sel0"], sl(hS, c, J1), start=False, stop=False)
                        nc.tensor.matmul(an, B["b_sel0"], sl(hSE, c, J1), start=False, stop=False)
                        nc.tensor.matmul(an, B["b_sel0"], sl(hSW, c, J1), start=False, stop=True)
                    else:
                        phS, phSE, phSW = prev_h["s"], prev_h["se"], prev_h["sw"]
                        nc.tensor.matmul(an, B["b_nsel"], sl(phS, c, J1), start=False, stop=False)
                        nc.tensor.matmul(an, B["b_nsel"], sl(phSE, c, J0), start=False, stop=False)
                        nc.tensor.matmul(an, B["b_nsel"], sl(phSW, c, J2), start=False, stop=True)

                prev_g, prev_h = g, h
                prev_evac = (den_ps, a_ps, pmid, r0)

    nc.compile()
    return nc


_NC_CACHE = None


def _get_nc():
    global _NC_CACHE
    if _NC_CACHE is None:
        _NC_CACHE = build()
    return _NC_CACHE


def kernel(batch_img: np.ndarray) -> np.ndarray:
    assert batch_img.shape == (8, C, H, W), batch_img.shape
    # host-side prep: fp16 + [H, C, W] layout per image
    x = np.ascontiguousarray(
        np.asarray(batch_img, dtype=np.float16).transpose(0, 2, 1, 3))
    nc = _get_nc()
    in_maps = [{"x": x[b]} for b in range(8)]
    r = run_bass_kernel_spmd(nc, in_maps, core_ids=list(range(8)))
    out = np.stack([r.results[b]["y"] for b in range(8)], axis=0)  # [8,H,C,W]
    return np.ascontiguousarray(out.transpose(0, 2, 1, 3)).astype(np.float32)


if __name__ == "__main__":
    rng = np.random.default_rng(0)
    img = rng.random((8, C, H, W), np.float32)
    y = kernel(img)
    print("ran ok", y.shape, y.dtype)
